# revision 20
# baseline (speedup 1.0000x reference)
"""Paged GQA decode attention on 8 TRN2 NeuronCores (raw Bacc, manual sems).

Sharding: tensor-parallel over kv heads (8 kv heads -> 8 cores). Core h gets
q heads 4h..4h+3 and kv head h. block_tables/context_lens/slot_mapping are
preprocessed on host into flat token-slot gather indices (replicated).

Per core, per group of 32 seqs (rows of the batch tile = 4 heads x 32 seqs):
  QK:   per seq, per 128-token chunk: indirect-gather K [tok,128] from the
        flat cache, PE-transpose -> K^T, DVE copy to SBUF, then a matmul with
        a zero-padded qT (cols 4*bb..4*bb+4 hold seq bb's scaled q^T) that
        accumulates scores into one PSUM tile [128, 2048].
  softmax: additive -1e30 mask, exp with fused row-sum, reciprocal, p *= 1/l
        (no max subtraction: scores are N(0,1)-scaled, |s| < ~6).
  PV:   PE-transpose p in [128,128] blocks (amortized over all 32 seqs),
        then per seq/chunk: indirect-gather V [tok,128] and accumulate
        out[4,128] = p^T.T @ V in PSUM; ACT copies to SBUF, DMA out.

All engine streams are hand-scheduled with one counting semaphore per engine
(PE/DVE/ACT, +1 per DMA ring) and cumulative wait_ge thresholds.
Invalid positions (>= context_len) use gather index 2^28 with bounds_check:
the DMA skips them (no bytes moved); masked scores -> exp 0.
"""

import numpy as np

import concourse.bass as bass
import concourse.bacc as bacc
import concourse.mybir as mybir
from concourse.bass_utils import run_bass_kernel_spmd

B, H, HKV, D = 64, 32, 8, 128
PAGE, PAGES_PER_SEQ, NUM_PAGES = 32, 64, 4096
SMAX = PAGES_PER_SEQ * PAGE  # 2048
NSLOTS = NUM_PAGES * PAGE  # 131072
SCALE = 0.08838834764831843
G = H // HKV  # 4 q heads per kv head
NCORES = 8
GROUPS = 2
GB = B // GROUPS  # 32 seqs per group
CHUNK = 128
NCHUNK = SMAX // CHUNK  # 16
INVALID_IDX = 1 << 28
NKV = 8  # k/v gather tile ring depth
NKT = 4  # kT sbuf ring
NPS = 2  # transpose psum ring
NOT = 4  # out tile ring

f32 = mybir.dt.float32
i32 = mybir.dt.int32
Exp = mybir.ActivationFunctionType.Exp
Copy = mybir.ActivationFunctionType.Copy


def build_nc(nrep=1):
    nc = bacc.Bacc()
    qTpad = nc.declare_dram_parameter("qTpad", [D, B * 128], f32, isOutput=False)
    kc = nc.declare_dram_parameter("kc", [NSLOTS, D], f32, isOutput=False)
    vc = nc.declare_dram_parameter("vc", [NSLOTS, D], f32, isOutput=False)
    tokidx = nc.declare_dram_parameter("tokidx", [128, B * NCHUNK], i32, isOutput=False)
    maskadd = nc.declare_dram_parameter("maskadd", [GROUPS, 128, SMAX], f32, isOutput=False)
    ident_in = nc.declare_dram_parameter("ident", [128, 128], f32, isOutput=False)
    out = nc.declare_dram_parameter("out", [B, G, D], f32, isOutput=True)

    # ---------------- schedule bookkeeping (python ints, build-time) -------
    # PE stream positions, per group offset
    CPB = max(1, 512 // CHUNK)  # chunk-columns per PSUM bank (f32)
    SEQI = 2 * NCHUNK  # PE instrs per seq in QK phase
    PE_PER_GROUP = SEQI * GB + NCHUNK + NCHUNK * GB

    def cnt_tr(g2, bb, j):  # k-transpose of chunk (bb, j)
        pos = bb * SEQI + (0 if j == 0 else 2 * j - 1)
        return g2 * PE_PER_GROUP + pos + 1

    def cnt_qk(g2, bb, j):
        pos = bb * SEQI + (2 * j + 2 if j < NCHUNK - 1 else SEQI - 1)
        return g2 * PE_PER_GROUP + pos + 1

    def cnt_ptr(g2, j):
        return g2 * PE_PER_GROUP + SEQI * GB + j + 1

    def cnt_pv(g2, bb, j):
        return g2 * PE_PER_GROUP + SEQI * GB + NCHUNK + bb * NCHUNK + j + 1

    NMEMSET = 2 * NKV
    DVE_PER_GROUP = GB * NCHUNK + 3 + NCHUNK  # 512 copies + add/recip/pmul + 16

    def cnt_kcp(g2, bb, j):
        return NMEMSET + g2 * DVE_PER_GROUP + bb * NCHUNK + j + 1

    def cnt_add(g2):
        return NMEMSET + g2 * DVE_PER_GROUP + GB * NCHUNK + 1

    def cnt_recip(g2):
        return cnt_add(g2) + 1

    def cnt_pmul(g2):
        return cnt_add(g2) + 2

    def cnt_ptcp(g2, j):
        return cnt_add(g2) + 3 + j

    ACT_PER_GROUP = 1 + GB

    def cnt_exp(g2):
        return g2 * ACT_PER_GROUP + 1

    def cnt_ocp(g2, bb):
        return g2 * ACT_PER_GROUP + 1 + bb + 1


    def ks_val(g2, bb, j):
        return 16 * (g2 * (GB * NCHUNK // NKV) + (bb * NCHUNK + j) // NKV + 1)

    vs_val = ks_val

    from contextlib import ExitStack

    with ExitStack() as ctx:
        identity = ctx.enter_context(nc.sbuf_tensor("identity", [128, 128], f32))
        idx_all = ctx.enter_context(nc.sbuf_tensor("idx_all", [128, B * NCHUNK], i32))
        qT_all = ctx.enter_context(nc.sbuf_tensor("qT_all", [D, B * 128], f32))
        mask0 = ctx.enter_context(nc.sbuf_tensor("mask0", [128, SMAX], f32))
        mask1 = ctx.enter_context(nc.sbuf_tensor("mask1", [128, SMAX], f32))
        s_t = ctx.enter_context(nc.sbuf_tensor("s_t", [128, SMAX], f32))
        p_t = ctx.enter_context(nc.sbuf_tensor("p_t", [128, SMAX], f32))
        l_t = ctx.enter_context(nc.sbuf_tensor("l_t", [128, 1], f32))
        rl_t = ctx.enter_context(nc.sbuf_tensor("rl_t", [128, 1], f32))
        scores_ps = ctx.enter_context(nc.psum_tensor("scores_ps", [128, SMAX], f32))
        PEs = ctx.enter_context(nc.semaphore("PEs"))
        DVEs = ctx.enter_context(nc.semaphore("DVEs"))
        ACTs = ctx.enter_context(nc.semaphore("ACTs"))
        IDENTs = ctx.enter_context(nc.semaphore("IDENTs"))
        IDXs = ctx.enter_context(nc.semaphore("IDXs"))
        QTs = ctx.enter_context(nc.semaphore("QTs"))
        MKs = [ctx.enter_context(nc.semaphore(f"MK{i}")) for i in range(GROUPS)]
        OS = [ctx.enter_context(nc.semaphore(f"OS{i}")) for i in range(NOT)]
        masks = [mask0, mask1]
        ktiles = [ctx.enter_context(nc.sbuf_tensor(f"ktile{i}", [128, D], f32)) for i in range(NKV)]
        vtiles = [ctx.enter_context(nc.sbuf_tensor(f"vtile{i}", [128, D], f32)) for i in range(NKV)]
        kTts = [ctx.enter_context(nc.sbuf_tensor(f"kTt{i}", [128, 128], f32)) for i in range(NKT)]
        pTts = [ctx.enter_context(nc.sbuf_tensor(f"pTt{i}", [128, 128], f32)) for i in range(NCHUNK)]
        otiles = [ctx.enter_context(nc.sbuf_tensor(f"ot{i}", [G, D], f32)) for i in range(NOT)]
        trps = [ctx.enter_context(nc.psum_tensor(f"trps{i}", [128, 128], f32)) for i in range(NPS)]
        pvps = [ctx.enter_context(nc.psum_tensor(f"pvps{i}", [G, D], f32)) for i in range(2)]
        KS = [ctx.enter_context(nc.semaphore(f"KS{i}")) for i in range(NKV)]
        VS = [ctx.enter_context(nc.semaphore(f"VS{i}")) for i in range(NKV)]

        # trackers filled during PE-schedule derivation (deterministic):
        # which DVE copy freed each transpose-psum bank, per use
        # global transpose index -> bank (k-transposes then p-transposes, per group)
        # Precompute psum-bank free chains:
        tr_seq = []  # list of ("k", g2, bb, j) / ("p", g2, j) in PE order
        for g2 in range(GROUPS * nrep):
            for bb in range(GB):
                for j in range(NCHUNK):
                    tr_seq.append(("k", g2, bb, j))
            for j in range(NCHUNK):
                tr_seq.append(("p", g2, j))
        bank_free_dve = {}  # index in tr_seq -> dve count to wait for
        for t, item in enumerate(tr_seq):
            if t >= NPS:
                prev = tr_seq[t - NPS]
                if prev[0] == "k":
                    bank_free_dve[t] = cnt_kcp(prev[1], prev[2], prev[3])
                else:
                    bank_free_dve[t] = cnt_ptcp(prev[1], prev[2])
        tr_index = {}
        for t, item in enumerate(tr_seq):
            tr_index[item] = t

        with nc.Block() as block:

            @block.sync
            def _(sync):
                sync.dma_start(out=identity[:, :], in_=ident_in[:, :]).then_inc(IDENTs, 16)
                sync.dma_start(out=idx_all[:, :], in_=tokidx[:, :]).then_inc(IDXs, 16)
                sync.dma_start(out=qT_all[:, :], in_=qTpad[:, :]).then_inc(QTs, 16)
                sync.dma_start(out=mask0[:, :], in_=maskadd[0]).then_inc(MKs[0], 16)
                sync.dma_start(out=mask1[:, :], in_=maskadd[1]).then_inc(MKs[1], 16)
                for g2 in range(GROUPS * nrep):
                    for bb in range(GB):
                        b = (g2 % GROUPS) * GB + bb
                        sync.wait_ge(ACTs, cnt_ocp(g2, bb))
                        sync.dma_start(
                            out=out[b], in_=otiles[(g2 * GB + bb) % NOT][:, :]
                        ).then_inc(OS[(g2 * GB + bb) % NOT], 16)

            @block.gpsimd
            def _(gpsimd):
                bc_reg = gpsimd.to_reg(NSLOTS - 1)
                gpsimd.wait_ge(IDXs, 16)  # indices loaded
                gpsimd.wait_ge(DVEs, NMEMSET)  # tile rings zeroed
                last_pe_wait = 0
                ktile_read = [0] * NKV  # PE count of transpose reading pass p
                vtile_read = [0] * NKV

                def kgather(g2, bb, j):
                    nonlocal last_pe_wait
                    slot = (bb * NCHUNK + j) % NKV
                    if ktile_read[slot] > last_pe_wait:
                        last_pe_wait = ktile_read[slot]
                        gpsimd.wait_ge(PEs, last_pe_wait)
                    gpsimd.indirect_dma_start(
                        out=ktiles[slot][:, :],
                        out_offset=None,
                        in_=kc[:, :],
                        in_offset=bass.IndirectOffsetOnAxis(
                            ap=idx_all[:, ((g2 % GROUPS) * GB + bb) * NCHUNK + j :][:, :1],
                            axis=0,
                        ),
                        bounds_check=bc_reg,
                        oob_is_err=False,
                    ).then_inc(KS[slot], 16)
                    ktile_read[slot] = cnt_tr(g2, bb, j)

                def vgather(g2, bb, j):
                    nonlocal last_pe_wait
                    slot = (bb * NCHUNK + j) % NKV
                    if vtile_read[slot] > last_pe_wait:
                        last_pe_wait = vtile_read[slot]
                        gpsimd.wait_ge(PEs, last_pe_wait)
                    gpsimd.indirect_dma_start(
                        out=vtiles[slot][:, :],
                        out_offset=None,
                        in_=vc[:, :],
                        in_offset=bass.IndirectOffsetOnAxis(
                            ap=idx_all[:, ((g2 % GROUPS) * GB + bb) * NCHUNK + j :][:, :1],
                            axis=0,
                        ),
                        bounds_check=bc_reg,
                        oob_is_err=False,
                    ).then_inc(VS[slot], 16)
                    vtile_read[slot] = cnt_pv(g2, bb, j)

                for g2 in range(GROUPS * nrep):
                    for bb in range(GB):
                        for j in range(NCHUNK):
                            kgather(g2, bb, j)
                    for bb in range(GB):
                        for j in range(NCHUNK):
                            vgather(g2, bb, j)

            @block.tensor
            def _(tensor):
                last = {"DVE": 0, "KS": [0] * NKV, "VS": [0] * NKV, "ACT": 0}

                def wait_dve(v):
                    if v > last["DVE"]:
                        last["DVE"] = v
                        tensor.wait_ge(DVEs, v)

                def wait_act(v):
                    if v > last["ACT"]:
                        last["ACT"] = v
                        tensor.wait_ge(ACTs, v)

                tensor.wait_ge(IDENTs, 16)
                tensor.wait_ge(QTs, 16)
                for g2 in range(GROUPS * nrep):
                    for bb in range(GB):
                        for j in range(NCHUNK):
                            # transpose chunk (bb, j)
                            slot = (bb * NCHUNK + j) % NKV
                            t = tr_index[("k", g2, bb, j)]
                            ksv = ks_val(g2, bb, j)
                            if ksv > last["KS"][slot]:
                                last["KS"][slot] = ksv
                                tensor.wait_ge(KS[slot], ksv)
                            if t in bank_free_dve:
                                wait_dve(bank_free_dve[t])
                            if g2 >= 1 and bb == 0 and j == 0:
                                # scores psum reused: previous group's mask-add done
                                wait_dve(cnt_add(g2 - 1))
                            nc.tensor.transpose(
                                out=trps[t % NPS][:, :],
                                in_=ktiles[slot][:, :],
                                identity=identity[:, :],
                            ).then_inc(PEs, 1)
                            if j > 0:
                                jq = j - 1
                                wait_dve(cnt_kcp(g2, bb, jq))
                                nc.tensor.matmul(
                                    out=scores_ps[:, jq * CHUNK : (jq + 1) * CHUNK],
                                    lhsT=qT_all[:, ((g2 % GROUPS) * GB + bb) * 128 : ((g2 % GROUPS) * GB + bb + 1) * 128],
                                    rhs=kTts[(bb * NCHUNK + jq) % NKT][:, :],
                                    start=(bb == 0 and jq % CPB == 0),
                                    stop=(bb == GB - 1 and jq % CPB == CPB - 1),
                                    skip_group_check=True,
                                ).then_inc(PEs, 1)
                        jq = NCHUNK - 1
                        wait_dve(cnt_kcp(g2, bb, jq))
                        nc.tensor.matmul(
                            out=scores_ps[:, jq * CHUNK : (jq + 1) * CHUNK],
                            lhsT=qT_all[:, ((g2 % GROUPS) * GB + bb) * 128 : ((g2 % GROUPS) * GB + bb + 1) * 128],
                            rhs=kTts[(bb * NCHUNK + jq) % NKT][:, :],
                            start=(bb == 0 and jq % CPB == 0),
                            stop=(bb == GB - 1 and jq % CPB == CPB - 1),
                            skip_group_check=True,
                        ).then_inc(PEs, 1)

                    # p transposes
                    for j in range(NCHUNK):
                        t = tr_index[("p", g2, j)]
                        wait_dve(cnt_pmul(g2))
                        if t in bank_free_dve:
                            wait_dve(bank_free_dve[t])
                        nc.tensor.transpose(
                            out=trps[t % NPS][:, :],
                            in_=p_t[:, j * CHUNK : (j + 1) * CHUNK],
                            identity=identity[:, :],
                        ).then_inc(PEs, 1)

                    # PV
                    for bb in range(GB):
                        for j in range(NCHUNK):
                            slot = (bb * NCHUNK + j) % NKV
                            vsv = vs_val(g2, bb, j)
                            if vsv > last["VS"][slot]:
                                last["VS"][slot] = vsv
                                tensor.wait_ge(VS[slot], vsv)
                            wait_dve(cnt_ptcp(g2, j))
                            if j == 0:
                                # pv psum bank freed by ACT copy two seqs ago
                                k = g2 * GB + bb - 2
                                if k >= 0:
                                    wait_act(cnt_ocp(k // GB, k % GB))
                            nc.tensor.matmul(
                                out=pvps[bb % 2][:, :],
                                lhsT=pTts[j][:, 4 * bb : 4 * bb + 4],
                                rhs=vtiles[slot][:, :],
                                start=(j == 0),
                                stop=(j == NCHUNK - 1),
                                skip_group_check=True,
                            ).then_inc(PEs, 1)

            @block.vector
            def _(vector):
                last = {"PE": 0, "ACT": 0}

                def wait_pe(v):
                    if v > last["PE"]:
                        last["PE"] = v
                        vector.wait_ge(PEs, v)

                def wait_act(v):
                    if v > last["ACT"]:
                        last["ACT"] = v
                        vector.wait_ge(ACTs, v)

                for ktile in ktiles:
                    vector.memset(ktile[:, :], 0.0).then_inc(DVEs, 1)
                for vtile in vtiles:
                    vector.memset(vtile[:, :], 0.0).then_inc(DVEs, 1)

                kTt_read = [0] * NKT
                for g2 in range(GROUPS * nrep):
                    for bb in range(GB):
                        for j in range(NCHUNK):
                            t = tr_index[("k", g2, bb, j)]
                            wait_pe(cnt_tr(g2, bb, j))
                            r = (bb * NCHUNK + j) % NKT
                            wait_pe(kTt_read[r])
                            nc.vector.tensor_copy(
                                out=kTts[r][:, :], in_=trps[t % NPS][:, :]
                            ).then_inc(DVEs, 1)
                            kTt_read[r] = cnt_qk(g2, bb, j)
                    # softmax
                    wait_pe(cnt_qk(g2, GB - 1, NCHUNK - 1))
                    vector.wait_ge(MKs[g2 % GROUPS], 16)
                    nc.vector.tensor_add(
                        out=s_t[:, :], in0=scores_ps[:, :], in1=masks[g2 % GROUPS][:, :]
                    ).then_inc(DVEs, 1)
                    wait_act(cnt_exp(g2))
                    nc.vector.reciprocal(out=rl_t[:, :], in_=l_t[:, :]).then_inc(DVEs, 1)
                    vector.wait_ge(DVEs, cnt_recip(g2))  # DVE pipeline RAW on rl_t
                    nc.vector.tensor_scalar_mul(
                        out=p_t[:, :], in0=p_t[:, :], scalar1=rl_t[:, :1]
                    ).then_inc(DVEs, 1)
                    for j in range(NCHUNK):
                        t = tr_index[("p", g2, j)]
                        wait_pe(cnt_ptr(g2, j))
                        if g2 > 0:
                            wait_pe(cnt_pv(g2 - 1, GB - 1, j))
                        nc.vector.tensor_copy(
                            out=pTts[j][:, :], in_=trps[t % NPS][:, :]
                        ).then_inc(DVEs, 1)

            @block.scalar
            def _(scalar):
                last = {"PE": 0, "DVE": 0}

                def wait_pe(v):
                    if v > last["PE"]:
                        last["PE"] = v
                        scalar.wait_ge(PEs, v)

                def wait_dve(v):
                    if v > last["DVE"]:
                        last["DVE"] = v
                        scalar.wait_ge(DVEs, v)

                for g2 in range(GROUPS * nrep):
                    wait_dve(cnt_add(g2))
                    if g2 > 0:
                        wait_pe(cnt_ptr(g2 - 1, NCHUNK - 1))  # p_t free
                    nc.scalar.activation(
                        out=p_t[:, :], in_=s_t[:, :], func=Exp, accum_out=l_t[:, :1]
                    ).then_inc(ACTs, 1)
                    for bb in range(GB):
                        wait_pe(cnt_pv(g2, bb, NCHUNK - 1))
                        r = (g2 * GB + bb) % NOT
                        k = g2 * GB + bb - NOT
                        if k >= 0:
                            scalar.wait_ge(OS[r], 16 * (k // NOT + 1))
                        nc.scalar.activation(
                            out=otiles[r][:, :], in_=pvps[bb % 2][:, :], func=Copy
                        ).then_inc(ACTs, 1)

    nc.compile()
    return nc


_NC_CACHE = None


def _get_nc():
    global _NC_CACHE
    if _NC_CACHE is None:
        _NC_CACHE = build_nc()
    return _NC_CACHE


def make_in_maps(q, k, v, k_cache, v_cache, block_tables, context_lens, slot_mapping):
    q = np.asarray(q, np.float32)
    k = np.asarray(k, np.float32)
    v = np.asarray(v, np.float32)
    k_cache = np.asarray(k_cache, np.float32)
    v_cache = np.asarray(v_cache, np.float32)
    block_tables = np.asarray(block_tables, np.int32)
    context_lens = np.asarray(context_lens, np.int32)
    slot_mapping = np.asarray(slot_mapping, np.int32)

    pos = np.arange(SMAX)
    page_ids = block_tables[:, pos // PAGE].astype(np.int64)  # [B, SMAX]
    slot_all = page_ids * PAGE + (pos % PAGE)[None, :]  # [B, SMAX]
    invalid = pos[None, :] >= context_lens[:, None]  # [B, SMAX]
    slot_all = np.where(invalid, INVALID_IDX, slot_all).astype(np.int32)
    tokidx = np.ascontiguousarray(
        slot_all.reshape(B, NCHUNK, CHUNK).transpose(2, 0, 1).reshape(CHUNK, B * NCHUNK)
    )  # [128, b*NCHUNK+j]

    m = np.where(invalid, np.float32(-1e30), np.float32(0.0)).astype(np.float32)
    maskadd = np.ascontiguousarray(
        np.repeat(m.reshape(GROUPS, GB, 1, SMAX), G, axis=2).reshape(
            GROUPS, GB * G, SMAX
        )
    )

    ident = np.eye(128, dtype=np.float32)

    bb = np.arange(B) % GB
    in_maps = []
    for h in range(NCORES):
        qh = q[:, G * h : G * h + G, :] * np.float32(SCALE)  # [B, 4, 128]
        qTpad3 = np.zeros((B, D, 128), np.float32)
        for g in range(G):
            qTpad3[np.arange(B), :, 4 * bb + g] = qh[:, g, :]
        qTpad = np.ascontiguousarray(qTpad3.transpose(1, 0, 2).reshape(D, B * 128))
        kc_h = np.ascontiguousarray(k_cache[:, :, h, :]).reshape(NSLOTS, D)
        vc_h = np.ascontiguousarray(v_cache[:, :, h, :]).reshape(NSLOTS, D)
        # store_kvcache: scatter the new per-seq token into the flat caches
        kc_h[slot_mapping] = k[:, h, :]
        vc_h[slot_mapping] = v[:, h, :]
        in_maps.append(
            {
                "qTpad": qTpad,
                "kc": kc_h,
                "vc": vc_h,
                "tokidx": tokidx,
                "maskadd": maskadd,
                "ident": ident,
            }
        )
    return in_maps


def kernel(q, k, v, k_cache, v_cache, block_tables, context_lens, slot_mapping,
           trace=False, **trace_kwargs):
    in_maps = make_in_maps(
        q, k, v, k_cache, v_cache, block_tables, context_lens, slot_mapping
    )
    nc = _get_nc()
    res = run_bass_kernel_spmd(
        nc, in_maps, core_ids=list(range(NCORES)), trace=trace, **trace_kwargs
    )
    outs = [res.results[i]["out"] for i in range(NCORES)]  # each [B, 4, 128]
    full = np.concatenate(outs, axis=1).astype(np.float32)  # [B, 32, 128]
    if trace:
        return full, res
    return full


# revision 21
# speedup vs baseline: 4.1073x; 4.1073x over previous
"""Paged GQA decode attention on 8 TRN2 NeuronCores (raw Bacc, manual sems).

Sharding: tensor-parallel over kv heads (8 kv heads -> 8 cores). Core h gets
q heads 4h..4h+3 and kv head h. block_tables/context_lens/slot_mapping are
preprocessed on host into flat token-slot gather indices (replicated).

Per core, per group of 32 seqs (rows of the batch tile = 4 heads x 32 seqs):
  QK:   per seq, per 128-token chunk: indirect-gather K [tok,128] from the
        flat cache, PE-transpose -> K^T, DVE copy to SBUF, then a matmul with
        a zero-padded qT (cols 4*bb..4*bb+4 hold seq bb's scaled q^T) that
        accumulates scores into one PSUM tile [128, 2048].
  softmax: additive -1e30 mask, exp with fused row-sum, reciprocal, p *= 1/l
        (no max subtraction: scores are N(0,1)-scaled, |s| < ~6).
  PV:   PE-transpose p in [128,128] blocks (amortized over all 32 seqs),
        then per seq/chunk: indirect-gather V [tok,128] and accumulate
        out[4,128] = p^T.T @ V in PSUM; ACT copies to SBUF, DMA out.

All engine streams are hand-scheduled with one counting semaphore per engine
(PE/DVE/ACT, +1 per DMA ring) and cumulative wait_ge thresholds.
Invalid positions (>= context_len) use gather index 2^28 with bounds_check:
the DMA skips them (no bytes moved); masked scores -> exp 0.
"""

import numpy as np

import concourse.bass as bass
import concourse.bacc as bacc
import concourse.mybir as mybir
from concourse.bass_utils import run_bass_kernel_spmd

B, H, HKV, D = 64, 32, 8, 128
PAGE, PAGES_PER_SEQ, NUM_PAGES = 32, 64, 4096
SMAX = PAGES_PER_SEQ * PAGE  # 2048
NSLOTS = NUM_PAGES * PAGE  # 131072
SCALE = 0.08838834764831843
G = H // HKV  # 4 q heads per kv head
NCORES = 8
GROUPS = 2
GB = B // GROUPS  # 32 seqs per group
CHUNK = 128
NCHUNK = SMAX // CHUNK  # 16
INVALID_IDX = 1 << 28
NKV = 8  # k/v gather tile ring depth
NKT = 4  # kT sbuf ring
NPS = 2  # transpose psum ring
NOT = 4  # out tile ring

f32 = mybir.dt.float32
i32 = mybir.dt.int32
Exp = mybir.ActivationFunctionType.Exp
Copy = mybir.ActivationFunctionType.Copy


def build_nc(nrep=1, no_compute=False, no_gather=False):
    nc = bacc.Bacc()
    qTpad = nc.declare_dram_parameter("qTpad", [D, B * 128], f32, isOutput=False)
    kc = nc.declare_dram_parameter("kc", [NSLOTS, D], f32, isOutput=False)
    vc = nc.declare_dram_parameter("vc", [NSLOTS, D], f32, isOutput=False)
    tokidx = nc.declare_dram_parameter("tokidx", [128, B * NCHUNK], i32, isOutput=False)
    maskadd = nc.declare_dram_parameter("maskadd", [GROUPS, 128, SMAX], f32, isOutput=False)
    ident_in = nc.declare_dram_parameter("ident", [128, 128], f32, isOutput=False)
    out = nc.declare_dram_parameter("out", [B, G, D], f32, isOutput=True)

    # ---------------- schedule bookkeeping (python ints, build-time) -------
    # PE stream positions, per group offset
    CPB = max(1, 512 // CHUNK)  # chunk-columns per PSUM bank (f32)
    SEQI = 2 * NCHUNK  # PE instrs per seq in QK phase
    PE_PER_GROUP = SEQI * GB + NCHUNK + NCHUNK * GB

    def cnt_tr(g2, bb, j):  # k-transpose of chunk (bb, j)
        pos = bb * SEQI + (0 if j == 0 else 2 * j - 1)
        return g2 * PE_PER_GROUP + pos + 1

    def cnt_qk(g2, bb, j):
        pos = bb * SEQI + (2 * j + 2 if j < NCHUNK - 1 else SEQI - 1)
        return g2 * PE_PER_GROUP + pos + 1

    def cnt_ptr(g2, j):
        return g2 * PE_PER_GROUP + SEQI * GB + j + 1

    def cnt_pv(g2, bb, j):
        return g2 * PE_PER_GROUP + SEQI * GB + NCHUNK + bb * NCHUNK + j + 1

    NMEMSET = 2 * NKV
    DVE_PER_GROUP = GB * NCHUNK + 3 + NCHUNK  # 512 copies + add/recip/pmul + 16

    def cnt_kcp(g2, bb, j):
        return NMEMSET + g2 * DVE_PER_GROUP + bb * NCHUNK + j + 1

    def cnt_add(g2):
        return NMEMSET + g2 * DVE_PER_GROUP + GB * NCHUNK + 1

    def cnt_recip(g2):
        return cnt_add(g2) + 1

    def cnt_pmul(g2):
        return cnt_add(g2) + 2

    def cnt_ptcp(g2, j):
        return cnt_add(g2) + 3 + j

    ACT_PER_GROUP = 1 + GB

    def cnt_exp(g2):
        return g2 * ACT_PER_GROUP + 1

    def cnt_ocp(g2, bb):
        return g2 * ACT_PER_GROUP + 1 + bb + 1


    def ks_val(g2, bb, j):
        return 16 * (g2 * (GB * NCHUNK // NKV) + (bb * NCHUNK + j) // NKV + 1)

    vs_val = ks_val

    from contextlib import ExitStack

    with ExitStack() as ctx:
        identity = ctx.enter_context(nc.sbuf_tensor("identity", [128, 128], f32))
        idx_all = ctx.enter_context(nc.sbuf_tensor("idx_all", [128, B * NCHUNK], i32))
        qT_all = ctx.enter_context(nc.sbuf_tensor("qT_all", [D, B * 128], f32))
        mask0 = ctx.enter_context(nc.sbuf_tensor("mask0", [128, SMAX], f32))
        mask1 = ctx.enter_context(nc.sbuf_tensor("mask1", [128, SMAX], f32))
        s_t = ctx.enter_context(nc.sbuf_tensor("s_t", [128, SMAX], f32))
        p_t = ctx.enter_context(nc.sbuf_tensor("p_t", [128, SMAX], f32))
        l_t = ctx.enter_context(nc.sbuf_tensor("l_t", [128, 1], f32))
        rl_t = ctx.enter_context(nc.sbuf_tensor("rl_t", [128, 1], f32))
        scores_ps = ctx.enter_context(nc.psum_tensor("scores_ps", [128, SMAX], f32))
        PEs = ctx.enter_context(nc.semaphore("PEs"))
        DVEs = ctx.enter_context(nc.semaphore("DVEs"))
        ACTs = ctx.enter_context(nc.semaphore("ACTs"))
        IDENTs = ctx.enter_context(nc.semaphore("IDENTs"))
        IDXs = ctx.enter_context(nc.semaphore("IDXs"))
        QTs = ctx.enter_context(nc.semaphore("QTs"))
        MKs = [ctx.enter_context(nc.semaphore(f"MK{i}")) for i in range(GROUPS)]
        OS = [ctx.enter_context(nc.semaphore(f"OS{i}")) for i in range(NOT)]
        masks = [mask0, mask1]
        ktiles = [ctx.enter_context(nc.sbuf_tensor(f"ktile{i}", [128, D], f32)) for i in range(NKV)]
        vtiles = [ctx.enter_context(nc.sbuf_tensor(f"vtile{i}", [128, D], f32)) for i in range(NKV)]
        kTts = [ctx.enter_context(nc.sbuf_tensor(f"kTt{i}", [128, 128], f32)) for i in range(NKT)]
        pTts = [ctx.enter_context(nc.sbuf_tensor(f"pTt{i}", [128, 128], f32)) for i in range(NCHUNK)]
        otiles = [ctx.enter_context(nc.sbuf_tensor(f"ot{i}", [G, D], f32)) for i in range(NOT)]
        trps = [ctx.enter_context(nc.psum_tensor(f"trps{i}", [128, 128], f32)) for i in range(NPS)]
        pvps = [ctx.enter_context(nc.psum_tensor(f"pvps{i}", [G, D], f32)) for i in range(2)]
        KS = [ctx.enter_context(nc.semaphore(f"KS{i}")) for i in range(NKV)]
        VS = [ctx.enter_context(nc.semaphore(f"VS{i}")) for i in range(NKV)]

        # trackers filled during PE-schedule derivation (deterministic):
        # which DVE copy freed each transpose-psum bank, per use
        # global transpose index -> bank (k-transposes then p-transposes, per group)
        # Precompute psum-bank free chains:
        tr_seq = []  # list of ("k", g2, bb, j) / ("p", g2, j) in PE order
        for g2 in range(GROUPS * nrep):
            for bb in range(GB):
                for j in range(NCHUNK):
                    tr_seq.append(("k", g2, bb, j))
            for j in range(NCHUNK):
                tr_seq.append(("p", g2, j))
        bank_free_dve = {}  # index in tr_seq -> dve count to wait for
        for t, item in enumerate(tr_seq):
            if t >= NPS:
                prev = tr_seq[t - NPS]
                if prev[0] == "k":
                    bank_free_dve[t] = cnt_kcp(prev[1], prev[2], prev[3])
                else:
                    bank_free_dve[t] = cnt_ptcp(prev[1], prev[2])
        tr_index = {}
        for t, item in enumerate(tr_seq):
            tr_index[item] = t

        with nc.Block() as block:

            @block.sync
            def _(sync):
                sync.dma_start(out=identity[:, :], in_=ident_in[:, :]).then_inc(IDENTs, 16)
                sync.dma_start(out=idx_all[:, :], in_=tokidx[:, :]).then_inc(IDXs, 16)
                sync.dma_start(out=qT_all[:, :], in_=qTpad[:, :]).then_inc(QTs, 16)
                sync.dma_start(out=mask0[:, :], in_=maskadd[0]).then_inc(MKs[0], 16)
                sync.dma_start(out=mask1[:, :], in_=maskadd[1]).then_inc(MKs[1], 16)
                for g2 in (range(0) if no_compute else range(GROUPS * nrep)):
                    for bb in range(GB):
                        b = (g2 % GROUPS) * GB + bb
                        sync.wait_ge(ACTs, cnt_ocp(g2, bb))
                        sync.dma_start(
                            out=out[b], in_=otiles[(g2 * GB + bb) % NOT][:, :]
                        ).then_inc(OS[(g2 * GB + bb) % NOT], 16)

            @block.gpsimd
            def _(gpsimd):
                bc_reg = gpsimd.to_reg(NSLOTS - 1)
                gpsimd.wait_ge(IDXs, 16)  # indices loaded
                gpsimd.wait_ge(DVEs, NMEMSET)  # tile rings zeroed
                last_pe_wait = 0
                ktile_read = [0] * NKV  # PE count of transpose reading pass p
                vtile_read = [0] * NKV

                def kgather(g2, bb, j):
                    nonlocal last_pe_wait
                    slot = (bb * NCHUNK + j) % NKV
                    if not no_compute and ktile_read[slot] > last_pe_wait:
                        last_pe_wait = ktile_read[slot]
                        gpsimd.wait_ge(PEs, last_pe_wait)
                    gpsimd.indirect_dma_start(
                        out=ktiles[slot][:, :],
                        out_offset=None,
                        in_=kc[:, :],
                        in_offset=bass.IndirectOffsetOnAxis(
                            ap=idx_all[:, ((g2 % GROUPS) * GB + bb) * NCHUNK + j :][:, :1],
                            axis=0,
                        ),
                        bounds_check=bc_reg,
                        oob_is_err=False,
                    ).then_inc(KS[slot], 16)
                    ktile_read[slot] = cnt_tr(g2, bb, j)

                def vgather(g2, bb, j):
                    nonlocal last_pe_wait
                    slot = (bb * NCHUNK + j) % NKV
                    if not no_compute and vtile_read[slot] > last_pe_wait:
                        last_pe_wait = vtile_read[slot]
                        gpsimd.wait_ge(PEs, last_pe_wait)
                    gpsimd.indirect_dma_start(
                        out=vtiles[slot][:, :],
                        out_offset=None,
                        in_=vc[:, :],
                        in_offset=bass.IndirectOffsetOnAxis(
                            ap=idx_all[:, ((g2 % GROUPS) * GB + bb) * NCHUNK + j :][:, :1],
                            axis=0,
                        ),
                        bounds_check=bc_reg,
                        oob_is_err=False,
                    ).then_inc(VS[slot], 16)
                    vtile_read[slot] = cnt_pv(g2, bb, j)

                if not no_gather:
                    for g2 in range(GROUPS * nrep):
                        for bb in range(GB):
                            for j in range(NCHUNK):
                                kgather(g2, bb, j)
                        for bb in range(GB):
                            for j in range(NCHUNK):
                                vgather(g2, bb, j)

            @block.tensor
            def _(tensor):
                last = {"DVE": 0, "KS": [0] * NKV, "VS": [0] * NKV, "ACT": 0}

                def wait_dve(v):
                    if v > last["DVE"]:
                        last["DVE"] = v
                        tensor.wait_ge(DVEs, v)

                def wait_act(v):
                    if v > last["ACT"]:
                        last["ACT"] = v
                        tensor.wait_ge(ACTs, v)

                tensor.wait_ge(IDENTs, 16)
                tensor.wait_ge(QTs, 16)
                for g2 in range([], range(GROUPS * nrep))[not no_compute] if False else (range(0) if no_compute else range(GROUPS * nrep)):
                    for bb in range(GB):
                        for j in range(NCHUNK):
                            # transpose chunk (bb, j)
                            slot = (bb * NCHUNK + j) % NKV
                            t = tr_index[("k", g2, bb, j)]
                            ksv = ks_val(g2, bb, j)
                            if not no_gather and ksv > last["KS"][slot]:
                                last["KS"][slot] = ksv
                                tensor.wait_ge(KS[slot], ksv)
                            if t in bank_free_dve:
                                wait_dve(bank_free_dve[t])
                            if g2 >= 1 and bb == 0 and j == 0:
                                # scores psum reused: previous group's mask-add done
                                wait_dve(cnt_add(g2 - 1))
                            nc.tensor.transpose(
                                out=trps[t % NPS][:, :],
                                in_=ktiles[slot][:, :],
                                identity=identity[:, :],
                            ).then_inc(PEs, 1)
                            if j > 0:
                                jq = j - 1
                                wait_dve(cnt_kcp(g2, bb, jq))
                                nc.tensor.matmul(
                                    out=scores_ps[:, jq * CHUNK : (jq + 1) * CHUNK],
                                    lhsT=qT_all[:, ((g2 % GROUPS) * GB + bb) * 128 : ((g2 % GROUPS) * GB + bb + 1) * 128],
                                    rhs=kTts[(bb * NCHUNK + jq) % NKT][:, :],
                                    start=(bb == 0 and jq % CPB == 0),
                                    stop=(bb == GB - 1 and jq % CPB == CPB - 1),
                                    skip_group_check=True,
                                ).then_inc(PEs, 1)
                        jq = NCHUNK - 1
                        wait_dve(cnt_kcp(g2, bb, jq))
                        nc.tensor.matmul(
                            out=scores_ps[:, jq * CHUNK : (jq + 1) * CHUNK],
                            lhsT=qT_all[:, ((g2 % GROUPS) * GB + bb) * 128 : ((g2 % GROUPS) * GB + bb + 1) * 128],
                            rhs=kTts[(bb * NCHUNK + jq) % NKT][:, :],
                            start=(bb == 0 and jq % CPB == 0),
                            stop=(bb == GB - 1 and jq % CPB == CPB - 1),
                            skip_group_check=True,
                        ).then_inc(PEs, 1)

                    # p transposes
                    for j in range(NCHUNK):
                        t = tr_index[("p", g2, j)]
                        wait_dve(cnt_pmul(g2))
                        if t in bank_free_dve:
                            wait_dve(bank_free_dve[t])
                        nc.tensor.transpose(
                            out=trps[t % NPS][:, :],
                            in_=p_t[:, j * CHUNK : (j + 1) * CHUNK],
                            identity=identity[:, :],
                        ).then_inc(PEs, 1)

                    # PV
                    for bb in range(GB):
                        for j in range(NCHUNK):
                            slot = (bb * NCHUNK + j) % NKV
                            vsv = vs_val(g2, bb, j)
                            if not no_gather and vsv > last["VS"][slot]:
                                last["VS"][slot] = vsv
                                tensor.wait_ge(VS[slot], vsv)
                            wait_dve(cnt_ptcp(g2, j))
                            if j == 0:
                                # pv psum bank freed by ACT copy two seqs ago
                                k = g2 * GB + bb - 2
                                if k >= 0:
                                    wait_act(cnt_ocp(k // GB, k % GB))
                            nc.tensor.matmul(
                                out=pvps[bb % 2][:, :],
                                lhsT=pTts[j][:, 4 * bb : 4 * bb + 4],
                                rhs=vtiles[slot][:, :],
                                start=(j == 0),
                                stop=(j == NCHUNK - 1),
                                skip_group_check=True,
                            ).then_inc(PEs, 1)

            @block.vector
            def _(vector):
                last = {"PE": 0, "ACT": 0}

                def wait_pe(v):
                    if v > last["PE"]:
                        last["PE"] = v
                        vector.wait_ge(PEs, v)

                def wait_act(v):
                    if v > last["ACT"]:
                        last["ACT"] = v
                        vector.wait_ge(ACTs, v)

                for ktile in ktiles:
                    vector.memset(ktile[:, :], 0.0).then_inc(DVEs, 1)
                for vtile in vtiles:
                    vector.memset(vtile[:, :], 0.0).then_inc(DVEs, 1)

                kTt_read = [0] * NKT
                for g2 in (range(0) if no_compute else range(GROUPS * nrep)):
                    for bb in range(GB):
                        for j in range(NCHUNK):
                            t = tr_index[("k", g2, bb, j)]
                            wait_pe(cnt_tr(g2, bb, j))
                            r = (bb * NCHUNK + j) % NKT
                            wait_pe(kTt_read[r])
                            nc.vector.tensor_copy(
                                out=kTts[r][:, :], in_=trps[t % NPS][:, :]
                            ).then_inc(DVEs, 1)
                            kTt_read[r] = cnt_qk(g2, bb, j)
                    # softmax
                    wait_pe(cnt_qk(g2, GB - 1, NCHUNK - 1))
                    vector.wait_ge(MKs[g2 % GROUPS], 16)
                    nc.vector.tensor_add(
                        out=s_t[:, :], in0=scores_ps[:, :], in1=masks[g2 % GROUPS][:, :]
                    ).then_inc(DVEs, 1)
                    wait_act(cnt_exp(g2))
                    nc.vector.reciprocal(out=rl_t[:, :], in_=l_t[:, :]).then_inc(DVEs, 1)
                    vector.wait_ge(DVEs, cnt_recip(g2))  # DVE pipeline RAW on rl_t
                    nc.vector.tensor_scalar_mul(
                        out=p_t[:, :], in0=p_t[:, :], scalar1=rl_t[:, :1]
                    ).then_inc(DVEs, 1)
                    for j in range(NCHUNK):
                        t = tr_index[("p", g2, j)]
                        wait_pe(cnt_ptr(g2, j))
                        if g2 > 0:
                            wait_pe(cnt_pv(g2 - 1, GB - 1, j))
                        nc.vector.tensor_copy(
                            out=pTts[j][:, :], in_=trps[t % NPS][:, :]
                        ).then_inc(DVEs, 1)

            @block.scalar
            def _(scalar):
                last = {"PE": 0, "DVE": 0}

                def wait_pe(v):
                    if v > last["PE"]:
                        last["PE"] = v
                        scalar.wait_ge(PEs, v)

                def wait_dve(v):
                    if v > last["DVE"]:
                        last["DVE"] = v
                        scalar.wait_ge(DVEs, v)

                for g2 in (range(0) if no_compute else range(GROUPS * nrep)):
                    wait_dve(cnt_add(g2))
                    if g2 > 0:
                        wait_pe(cnt_ptr(g2 - 1, NCHUNK - 1))  # p_t free
                    nc.scalar.activation(
                        out=p_t[:, :], in_=s_t[:, :], func=Exp, accum_out=l_t[:, :1]
                    ).then_inc(ACTs, 1)
                    for bb in range(GB):
                        wait_pe(cnt_pv(g2, bb, NCHUNK - 1))
                        r = (g2 * GB + bb) % NOT
                        k = g2 * GB + bb - NOT
                        if k >= 0:
                            scalar.wait_ge(OS[r], 16 * (k // NOT + 1))
                        nc.scalar.activation(
                            out=otiles[r][:, :], in_=pvps[bb % 2][:, :], func=Copy
                        ).then_inc(ACTs, 1)

    nc.compile()
    return nc


_NC_CACHE = None


def _get_nc():
    global _NC_CACHE
    if _NC_CACHE is None:
        _NC_CACHE = build_nc()
    return _NC_CACHE


def make_in_maps(q, k, v, k_cache, v_cache, block_tables, context_lens, slot_mapping):
    q = np.asarray(q, np.float32)
    k = np.asarray(k, np.float32)
    v = np.asarray(v, np.float32)
    k_cache = np.asarray(k_cache, np.float32)
    v_cache = np.asarray(v_cache, np.float32)
    block_tables = np.asarray(block_tables, np.int32)
    context_lens = np.asarray(context_lens, np.int32)
    slot_mapping = np.asarray(slot_mapping, np.int32)

    pos = np.arange(SMAX)
    page_ids = block_tables[:, pos // PAGE].astype(np.int64)  # [B, SMAX]
    slot_all = page_ids * PAGE + (pos % PAGE)[None, :]  # [B, SMAX]
    invalid = pos[None, :] >= context_lens[:, None]  # [B, SMAX]
    slot_all = np.where(invalid, INVALID_IDX, slot_all).astype(np.int32)
    tokidx = np.ascontiguousarray(
        slot_all.reshape(B, NCHUNK, CHUNK).transpose(2, 0, 1).reshape(CHUNK, B * NCHUNK)
    )  # [128, b*NCHUNK+j]

    m = np.where(invalid, np.float32(-1e30), np.float32(0.0)).astype(np.float32)
    maskadd = np.ascontiguousarray(
        np.repeat(m.reshape(GROUPS, GB, 1, SMAX), G, axis=2).reshape(
            GROUPS, GB * G, SMAX
        )
    )

    ident = np.eye(128, dtype=np.float32)

    bb = np.arange(B) % GB
    in_maps = []
    for h in range(NCORES):
        qh = q[:, G * h : G * h + G, :] * np.float32(SCALE)  # [B, 4, 128]
        qTpad3 = np.zeros((B, D, 128), np.float32)
        for g in range(G):
            qTpad3[np.arange(B), :, 4 * bb + g] = qh[:, g, :]
        qTpad = np.ascontiguousarray(qTpad3.transpose(1, 0, 2).reshape(D, B * 128))
        kc_h = np.ascontiguousarray(k_cache[:, :, h, :]).reshape(NSLOTS, D)
        vc_h = np.ascontiguousarray(v_cache[:, :, h, :]).reshape(NSLOTS, D)
        # store_kvcache: scatter the new per-seq token into the flat caches
        kc_h[slot_mapping] = k[:, h, :]
        vc_h[slot_mapping] = v[:, h, :]
        in_maps.append(
            {
                "qTpad": qTpad,
                "kc": kc_h,
                "vc": vc_h,
                "tokidx": tokidx,
                "maskadd": maskadd,
                "ident": ident,
            }
        )
    return in_maps


def kernel(q, k, v, k_cache, v_cache, block_tables, context_lens, slot_mapping,
           trace=False, **trace_kwargs):
    in_maps = make_in_maps(
        q, k, v, k_cache, v_cache, block_tables, context_lens, slot_mapping
    )
    nc = _get_nc()
    res = run_bass_kernel_spmd(
        nc, in_maps, core_ids=list(range(NCORES)), trace=trace, **trace_kwargs
    )
    outs = [res.results[i]["out"] for i in range(NCORES)]  # each [B, 4, 128]
    full = np.concatenate(outs, axis=1).astype(np.float32)  # [B, 32, 128]
    if trace:
        return full, res
    return full
